# revision 1
# baseline (speedup 1.0000x reference)
"""Trainium2 Bass kernel for nn_CausalSelfAttention_40810779247124.

Head-sharded (tensor-parallel) causal self-attention prefill across 8
NeuronCores: 2 heads per core. Per core:

  phase 1: QKV projection for its 2 heads, outputs in [e, tok] layout
           (contraction-friendly), Q/K kept resident in SBUF, V
           PE-transposed to [tok, e] and kept resident in SBUF.
  phase 2: attention computed transposed: scoresT[t,s] = K.T @ Q (both
           operands already have Dh on partitions), exp on ScalarE,
           denominator via ones-matmul (partition-sum on PE),
           wvT[Dh,s] = V.T @ P.T accumulated on PE.  wvT staged to DRAM.
  phase 3: output projection partial: out[tok,:] += wvT.T @ w_outT for
           this core's d-slice.  The all-reduce over cores is done on
           the host during unsharding (sum of 8 partials).

Causality is exploited (t>s blocks skipped); the host verifies that
mask/cache_pos match the causal-prefill pattern and falls back to a
numpy reference otherwise.  All matmul operands use float32r (full-rate
fp32 matmul mode on TRN2).
"""

import sys

sys.path.insert(0, "/opt/trn_rl_repo")

import numpy as np

B = 2
S = 2048
T = 4096
NS = 2048          # n_state
H = 16
DH = 128
NCORES = 8
HPC = H // NCORES  # heads per core = 2
DPC = HPC * DH     # d-slice per core = 256
TOK = B * S        # 4096 tokens across batches
SCALE = 1.0 / float(np.sqrt(DH))

_CACHED = {}


def _build_program():
    import concourse.bacc as bacc
    import concourse.bass as bass
    import concourse.tile as tile
    from concourse import mybir
    f32r = mybir.dt.float32r
    f32 = mybir.dt.float32

    nc = bacc.Bacc()

    xT = nc.dram_tensor("xT", [NS, TOK], f32r, kind="ExternalInput")
    wT = nc.dram_tensor("wT", [NS, 6 * DH], f32r, kind="ExternalInput")
    woutT = nc.dram_tensor("woutT", [DPC, NS], f32r, kind="ExternalInput")
    cmask = nc.dram_tensor("cmask", [DH, 4 * 512 + 256], f32r, kind="ExternalInput")
    outp = nc.dram_tensor("outp", [TOK, NS], f32, kind="ExternalOutput")

    NT = TOK // 512   # 8 tok-tiles of 512
    NK = NS // 128    # 16 contraction chunks

    with tile.TileContext(nc) as tc:
        with (
            tc.tile_pool(name="constp", bufs=1) as constp,
            tc.tile_pool(name="vresp", bufs=1) as vresp,
            tc.tile_pool(name="dramp", bufs=1, space="DRAM") as dramp,
        ):
            cmask_sb = constp.tile([DH, 4 * 512 + 256], f32r)
            ones_col = cmask_sb[:, 2176:2177]
            ones_row = cmask_sb[0:1, 2176:2304]

            # V resident across phases 1-2: v_res[p, c, e] = V[c*128+p, e]
            v_res = vresp.tile([128, TOK // 128, DPC], f32r)

            # wvT staging through DRAM between phases 2 and 3
            wvn_d = dramp.tile([B * DPC, S], f32r)

            with tc.tile_pool(name="qkresp", bufs=1) as qkresp:
                # Q,K resident [e-block(q0,q1,k0,k1), tok]
                qk_res = qkresp.tile([128, 4, TOK], f32r)

                # ---------------- phase 1: QKV projection ----------------
                with (
                    tc.tile_pool(name="wp", bufs=1) as wp,
                    tc.tile_pool(name="xp", bufs=3) as xp,
                    tc.tile_pool(name="qkv_ps", bufs=4, space="PSUM") as qkv_ps,
                    tc.tile_pool(name="v_ps", bufs=4, space="PSUM") as v_ps,
                ):
                    w_sb = wp.tile([128, NK, 6 * DH], f32r)

                    for a in range(NT):
                        pss = [
                            qkv_ps.tile([128, 512], f32, tag="qkv", name=f"qkv{m}")
                            for m in range(4)
                        ]
                        vps = [
                            v_ps.tile([128, 256], f32, tag="vps", name=f"vps{t}")
                            for t in range(4)
                        ]
                        for half in range(2):
                            x_sb = xp.tile([128, NK // 2, 512], f32r, tag="x_sb")
                            for kc in range(NK // 2):
                                nc.scalar.dma_start(
                                    out=x_sb[:, kc, :],
                                    in_=xT[
                                        1024 * half + 128 * kc : 1024 * half
                                        + 128 * (kc + 1),
                                        512 * a : 512 * (a + 1),
                                    ],
                                )
                            for kc in range(NK // 2):
                                kk = half * (NK // 2) + kc
                                if a == 0:
                                    if kk == 0:
                                        for mm in range(6):
                                            nc.sync.dma_start(
                                                out=w_sb[
                                                    :, kk, 128 * mm : 128 * (mm + 1)
                                                ],
                                                in_=wT[
                                                    128 * kk : 128 * (kk + 1),
                                                    128 * mm : 128 * (mm + 1),
                                                ],
                                            )
                                    else:
                                        nc.sync.dma_start(
                                            out=w_sb[:, kk, :],
                                            in_=wT[128 * kk : 128 * (kk + 1), :],
                                        )
                                for m in range(4):
                                    nc.tensor.matmul(
                                        pss[m],
                                        w_sb[:, kk, 128 * m : 128 * (m + 1)],
                                        x_sb[:, kc, :],
                                        start=(kk == 0),
                                        stop=(kk == NK - 1),
                                    )
                                for t in range(4):
                                    nc.tensor.matmul(
                                        vps[t],
                                        x_sb[:, kc, 128 * t : 128 * (t + 1)],
                                        w_sb[:, kk, 512:768],
                                        start=(kk == 0),
                                        stop=(kk == NK - 1),
                                    )
                        for m in range(4):
                            # Q/K to resident SBUF in [e, tok] layout
                            nc.vector.tensor_copy(
                                out=qk_res[:, m, 512 * a : 512 * (a + 1)],
                                in_=pss[m],
                            )
                        for t in range(4):
                            nc.vector.tensor_copy(
                                out=v_res[:, 4 * a + t, :], in_=vps[t]
                            )

                    nc.scalar.dma_start(
                        out=cmask_sb[:, 0:2048], in_=cmask[:, 0:2048]
                    )
                    nc.scalar.dma_start(
                        out=cmask_sb[:, 2176:2304], in_=cmask[:, 2176:2304]
                    )

                # ------- phases 2+3: attention + out-projection per batch -------
                with (
                    tc.tile_pool(name="woutp", bufs=1) as woutp,
                    tc.tile_pool(name="ptp", bufs=4) as ptp,
                    tc.tile_pool(name="zrp", bufs=2) as zrp,
                    tc.tile_pool(name="wvnp", bufs=5) as wvnp,
                    tc.tile_pool(name="ostage", bufs=3) as ostage,
                    tc.tile_pool(name="sc_ps", bufs=2, space="PSUM") as sc_ps,
                    tc.tile_pool(name="wv_ps", bufs=2, space="PSUM") as wv_ps,
                    tc.tile_pool(name="z_ps", bufs=2, space="PSUM") as z_ps,
                    tc.tile_pool(name="o_ps", bufs=2, space="PSUM") as o_ps,
                ):
                    wout_sb = woutp.tile([128, HPC, NS], f32r)
                    for h in range(HPC):
                        nc.sync.dma_start(
                            out=wout_sb[:, h, :],
                            in_=woutT[128 * h : 128 * (h + 1), :],
                        )
                    def finalize(fin):
                        wv, z, wvn, ast = fin
                        zr = zrp.tile([1, 512], f32r, tag="zr")
                        with nc.allow_low_precision(
                            reason="f32r is bit-identical to f32"
                        ):
                            nc.vector.reciprocal(out=zr, in_=z)
                        zb = z_ps.tile([128, 512], f32, tag="z")
                        nc.tensor.matmul(zb, ones_row, zr, start=True, stop=True)
                        zbs = zrp.tile([128, 512], f32r, tag="zbs")
                        nc.vector.tensor_copy(out=zbs, in_=zb)
                        nc.vector.tensor_mul(
                            wvn[:, 512 * ast : 512 * (ast + 1)], wv, zbs
                        )

                    for b in range(B):
                        wvn_tiles = []
                        for h in range(HPC):
                            q_sb = qk_res[:, h, S * b : S * (b + 1)]
                            k_sb = qk_res[:, 2 + h, S * b : S * (b + 1)]
                            wvn = wvnp.tile([128, S], f32r, tag="wvn")
                            wvn_tiles.append(wvn)
                            for ast in range(S // 512):
                                nj = 4 * ast + 4  # causal t-blocks
                                wv = wv_ps.tile([128, 512], f32, tag="wv")
                                z_full = z_ps.tile([128, 512], f32, tag="z")
                                z = z_full[0:1, :]
                                for j in range(nj):
                                    sc = sc_ps.tile([128, 512], f32, tag="sc")
                                    nc.tensor.matmul(
                                        sc,
                                        k_sb[:, 128 * j : 128 * (j + 1)],
                                        q_sb[:, 512 * ast : 512 * (ast + 1)],
                                        start=True,
                                        stop=True,
                                    )
                                    pt = ptp.tile([128, 512], f32r, tag="pt")
                                    nc.scalar.activation(
                                        out=pt,
                                        in_=sc,
                                        func=mybir.ActivationFunctionType.Exp,
                                        scale=SCALE,
                                    )
                                    p = j - 4 * ast
                                    if p >= 0:
                                        nc.vector.tensor_mul(
                                            pt,
                                            pt,
                                            cmask_sb[:, 512 * p : 512 * (p + 1)],
                                        )
                                    nc.tensor.matmul(
                                        z,
                                        ones_col,
                                        pt,
                                        start=(j == 0),
                                        stop=(j == nj - 1),
                                    )
                                    nc.tensor.matmul(
                                        wv,
                                        v_res[
                                            :, 16 * b + j, 128 * h : 128 * (h + 1)
                                        ],
                                        pt,
                                        start=(j == 0),
                                        stop=(j == nj - 1),
                                    )
                                finalize((wv, z, wvn, ast))
                        # out-projection for this batch (wvn of both heads)
                        for tk in range(S // 128):
                            ost = ostage.tile([128, NS], f32, tag="ost")
                            for n in range(NS // 512):
                                ops = o_ps.tile([128, 512], f32, tag="ops")
                                for h in range(HPC):
                                    nc.tensor.matmul(
                                        ops,
                                        wvn_tiles[h][:, 128 * tk : 128 * (tk + 1)],
                                        wout_sb[:, h, 512 * n : 512 * (n + 1)],
                                        start=(h == 0),
                                        stop=(h == HPC - 1),
                                    )
                                nc.vector.tensor_copy(
                                    out=ost[:, 512 * n : 512 * (n + 1)], in_=ops
                                )
                            for hh in range(2):
                                nc.sync.dma_start(
                                    out=outp[
                                        S * b + 128 * tk : S * b + 128 * (tk + 1),
                                        1024 * hh : 1024 * (hh + 1),
                                    ],
                                    in_=ost[:, 1024 * hh : 1024 * (hh + 1)],
                                )

    nc.compile()
    return nc


def _causal_fastpath_ok(mask, cache_pos):
    if cache_pos.shape != (S,) or not np.array_equal(
        np.asarray(cache_pos), np.arange(S, dtype=np.int64).astype(cache_pos.dtype)
    ):
        return False
    m = np.asarray(mask).reshape(S, T)
    rows = np.arange(S)[:, None]
    cols = np.arange(T)[None, :]
    return np.array_equal(m, cols <= rows)


def _numpy_fallback(input_ids, mask, cache_pos, w_qkv, w_out, k_cache, v_cache):
    x = np.asarray(input_ids, dtype=np.float32)
    qkv = np.einsum("bsd,ed->bse", x, np.asarray(w_qkv, np.float32))
    q, k, v = np.split(qkv, 3, axis=-1)

    def heads(t):
        return t.reshape(B, S, H, DH).transpose(0, 2, 1, 3)

    q, k, v = heads(q), heads(k), heads(v)
    kf = np.array(k_cache, np.float32)
    vf = np.array(v_cache, np.float32)
    kf[:, :, np.asarray(cache_pos)] = k
    vf[:, :, np.asarray(cache_pos)] = v
    sc = np.einsum("bhsd,bhtd->bhst", q, kf) * SCALE
    sc = np.where(np.asarray(mask), sc, np.finfo(np.float32).min)
    sc = sc - sc.max(axis=-1, keepdims=True)
    p = np.exp(sc)
    p = p / p.sum(axis=-1, keepdims=True)
    wv = np.einsum("bhst,bhtd->bhsd", p, vf)
    wv = wv.transpose(0, 2, 1, 3).reshape(B, S, NS)
    return np.einsum("bsd,ed->bse", wv, np.asarray(w_out, np.float32))


def _build_cmask_host():
    # 4 multiplicative mask tiles [128, 512] laid side by side: tile p is
    # applied to scoresT block (t rows) against an s-tile of width 512 when
    # the t-block is the p-th 128-strip inside that s-tile.
    t = np.arange(128)[:, None]
    s = np.arange(512)[None, :]
    tiles = []
    for p in range(4):
        tiles.append(((s - 128 * p) >= t).astype(np.float32))
    # trailing constant blocks: [identity(128) | ones(128)]
    tiles.append(np.eye(128, dtype=np.float32))
    tiles.append(np.ones((128, 128), dtype=np.float32))
    return np.concatenate(tiles, axis=1)  # [128, 2304]


def _run_on_device(in_maps, trace=False):
    from concourse.bass_utils import run_bass_kernel_spmd

    if "nc" not in _CACHED:
        _CACHED["nc"] = _build_program()
    nc = _CACHED["nc"]
    return run_bass_kernel_spmd(
        nc, in_maps, core_ids=list(range(NCORES)), trace=trace
    )


def _prep_in_maps(input_ids, w_qkv, w_out):
    x2d = np.ascontiguousarray(
        np.asarray(input_ids, np.float32).reshape(TOK, NS).T
    )  # [NS, TOK]
    cm = _build_cmask_host()
    wq = np.asarray(w_qkv, np.float32)
    wo = np.asarray(w_out, np.float32)
    in_maps = []
    for c in range(NCORES):
        lo, hi = c * DPC, (c + 1) * DPC
        w_slice = np.concatenate(
            [wq[lo:hi], wq[NS + lo : NS + hi], wq[2 * NS + lo : 2 * NS + hi]],
            axis=0,
        )  # [768, NS] (q,k,v rows for this core's heads)
        wT_c = np.ascontiguousarray(w_slice.T)        # [NS, 768]
        woutT_c = np.ascontiguousarray(wo[:, lo:hi].T)  # [DPC, NS]
        in_maps.append({"xT": x2d, "wT": wT_c, "woutT": woutT_c, "cmask": cm})
    return in_maps


def kernel(input_ids, mask, cache_pos, w_qkv, w_out, k_cache, v_cache):
    if not _causal_fastpath_ok(mask, cache_pos):
        return _numpy_fallback(
            input_ids, mask, cache_pos, w_qkv, w_out, k_cache, v_cache
        )
    in_maps = _prep_in_maps(input_ids, w_qkv, w_out)
    res = _run_on_device(in_maps)
    out = np.zeros((TOK, NS), np.float32)
    for r in res.results:
        out += r["outp"]
    return out.reshape(B, S, NS)



# revision 21
# speedup vs baseline: 1.2043x; 1.2043x over previous
"""Trainium2 Bass kernel for nn_CausalSelfAttention_40810779247124.

Head-sharded (tensor-parallel) causal self-attention prefill across 8
NeuronCores: 2 heads per core.  v2 — PE-roofline oriented rewrite:

  * all matmul operands in bf16 (full-rate at any free size, halves DMA
    bytes and SBUF residency; rel-err budget 2e-2 >> bf16's ~0.3%)
  * causal diagonal blocks trimmed: score/exp/wv work shrinks to the
    true 128-granular triangle (t-strip j vs 512-wide s-tile only
    computes the valid column range)
  * softmax denominator moved off the PE: per-strip partial sums
    accumulate on GpSimd (tensor_add), single ones-matmul per s-tile
    reduces partitions, so PE z-cost drops 160 matmuls -> 16/core
  * phase-1 QKV restructured block-at-a-time (2 concurrent PSUM banks
    instead of 8) so attention + out-projection PSUM pools coexist and
    all three phases pipeline: P1(b1) fills PE while attention(b0) is
    paced by ScalarE exps, out-proj(unit k-1) fills attention(unit k)
  * out-projection copies split DVE/GpSimd, output DMA on SP, x DMA on
    ScalarE -> no engine above ~55% of PE busy time

Per core: QKV projection for its 2 heads -> Q,K resident [e,tok] bf16,
V resident [tok,e] bf16; scoresT[t,s] = K.T @ Q on the PE, exp on
ScalarE, mask-mul on DVE (diagonal blocks only), wvT accumulated on the
PE; normalization via GpSimd-summed denominator + reciprocal +
ones-broadcast; out-projection partials summed across cores on the
host during unsharding.
"""

import sys

sys.path.insert(0, "/opt/trn_rl_repo")

import numpy as np

B = 2
S = 2048
T = 4096
NS = 2048          # n_state
H = 16
DH = 128
NCORES = 8
HPC = H // NCORES  # heads per core = 2
DPC = HPC * DH     # d-slice per core = 256
TOK = B * S        # 4096 tokens across batches
NK = NS // 128     # 16 contraction chunks
SCALE = 1.0 / float(np.sqrt(DH))

_CACHED = {}


def _build_program():
    import concourse.bacc as bacc
    import concourse.tile as tile
    from concourse import mybir

    bf16 = mybir.dt.bfloat16
    f32 = mybir.dt.float32
    f32r = mybir.dt.float32r
    Exp = mybir.ActivationFunctionType.Exp

    nc = bacc.Bacc()

    xT = nc.dram_tensor("xT", [NS, TOK], bf16, kind="ExternalInput")
    wT = nc.dram_tensor("wT", [NS, 6 * DH], bf16, kind="ExternalInput")
    woutT = nc.dram_tensor("woutT", [DPC, NS], f32r, kind="ExternalInput")
    # [causal mask 128 | zeros 384 | ones 1] in bf16
    cmask = nc.dram_tensor("cmask", [DH, DH + 385], bf16, kind="ExternalInput")
    onesf = nc.dram_tensor("onesf", [1, DH], f32r, kind="ExternalInput")
    outp = nc.dram_tensor("outp", [TOK, NS], f32, kind="ExternalOutput")

    with tile.TileContext(nc) as tc:
        with (
            tc.tile_pool(name="constp", bufs=1) as constp,
            tc.tile_pool(name="resp", bufs=1) as resp,
            tc.tile_pool(name="xp", bufs=2) as xp,
            tc.tile_pool(name="ptp", bufs=8) as ptp,
            tc.tile_pool(name="zrp", bufs=1) as zrp,
            tc.tile_pool(name="zbsp", bufs=2) as zbsp,
            tc.tile_pool(name="wvnp", bufs=4) as wvnp,
            tc.tile_pool(name="ostp", bufs=4) as ostp,
            tc.tile_pool(name="genp", bufs=3, space="PSUM") as genp,
            tc.tile_pool(name="scp", bufs=2, space="PSUM") as scp,
            tc.tile_pool(name="wvp", bufs=2, space="PSUM") as wvp,
            tc.tile_pool(name="zzp", bufs=1, space="PSUM") as zzp,
        ):
            cz_sb = constp.tile([DH, DH + 385], bf16, name="cz_sb")
            cmask_sb = cz_sb[:, 0:DH]
            zeros_sb = cz_sb[:, DH : DH + 384]
            ones_col = cz_sb[:, DH + 384 : DH + 385]
            ones_row = constp.tile([1, DH], f32r, name="ones_row")
            qk_res = resp.tile([128, 4, TOK], bf16)       # q0,q1,k0,k1
            v_flat = resp.tile([128, (TOK // DH) * DPC], bf16)  # [t, chunk*e]
            w_sb = resp.tile([128, NK, 6 * DH], bf16)
            wout_sb = resp.tile([128, HPC, NS], f32r)

            nc.sync.dma_start(out=cz_sb, in_=cmask[:, :])
            nc.sync.dma_start(out=ones_row, in_=onesf[:, :])
            for kk in range(NK):
                nc.sync.dma_start(
                    out=w_sb[:, kk, :], in_=wT[128 * kk : 128 * (kk + 1), :]
                )
            for h in range(HPC):
                nc.sync.dma_start(
                    out=wout_sb[:, h, :], in_=woutT[128 * h : 128 * (h + 1), :]
                )

            wvn_tiles = {}

            def p1(A):
                # QKV projection for tokens [1024A, 1024A+1024)
                x_sb = xp.tile([128, NK, 1024], bf16, tag="x", name="x_sb")
                for kc in range(NK):
                    nc.scalar.dma_start(
                        out=x_sb[:, kc, :],
                        in_=xT[128 * kc : 128 * (kc + 1), 1024 * A : 1024 * (A + 1)],
                    )
                for sub in range(2):
                    tb = 1024 * A + 512 * sub
                    for m in range(4):  # q0,q1,k0,k1 -> [e, tok]
                        ps = genp.tile([128, 512], f32, tag="gen", name="p1qk")
                        for kk in range(NK):
                            nc.tensor.matmul(
                                ps,
                                w_sb[:, kk, 128 * m : 128 * (m + 1)],
                                x_sb[:, kk, 512 * sub : 512 * (sub + 1)],
                                start=(kk == 0),
                                stop=(kk == NK - 1),
                            )
                        nc.vector.tensor_copy(
                            out=qk_res[:, m, tb : tb + 512], in_=ps
                        )
                    for vb in range(2):  # v chunk pairs -> [t, e]
                        ps = genp.tile([128, 512], f32, tag="gen", name="p1v")
                        for kk in range(NK):
                            for ci in range(2):
                                cl = 4 * sub + 2 * vb + ci
                                # both chunks form one bank-granular
                                # accumulation group (zero regions are
                                # per-bank): single start, single stop
                                nc.tensor.matmul(
                                    ps[:, 256 * ci : 256 * (ci + 1)],
                                    x_sb[:, kk, 128 * cl : 128 * (cl + 1)],
                                    w_sb[:, kk, 512:768],
                                    start=(kk == 0 and ci == 0),
                                    stop=(kk == NK - 1 and ci == 1),
                                    skip_group_check=True,
                                )
                        c0 = 8 * A + 4 * sub + 2 * vb
                        nc.scalar.copy(
                            out=v_flat[:, 256 * c0 : 256 * (c0 + 2)], in_=ps
                        )

            def att(b, h, ast):
                # attention for s-tile ast of (batch b, local head h)
                qb = S * b
                nj = 4 * ast + 4
                if (b, h) not in wvn_tiles:
                    wvn_tiles[(b, h)] = wvnp.tile(
                        [128, S], f32r, tag="wvn", name=f"wvn_{b}_{h}"
                    )
                wv = wvp.tile([128, 512], f32, tag="wv", name="wv")
                # z accumulated as a bf16 pairwise tree (log-depth rounding)
                zstack = []  # (level, partial-sum tile)
                for j in range(nj):
                    p = j - 4 * ast  # >= 0 on causal diagonal
                    o = 128 * p if p > 0 else 0
                    sc = scp.tile([128, 512], f32, tag="sc", name="sc")
                    nc.tensor.matmul(
                        sc[:, o:512],
                        qk_res[:, 2 + h, qb + 128 * j : qb + 128 * (j + 1)],
                        qk_res[:, h, qb + 512 * ast + o : qb + 512 * (ast + 1)],
                        start=True,
                        stop=True,
                    )
                    pt = ptp.tile([128, 512], bf16, tag="pt", name="pt")
                    if o > 0:
                        nc.gpsimd.tensor_copy(out=pt[:, 0:o], in_=zeros_sb[:, 0:o])
                    nc.scalar.activation(
                        out=pt[:, o:512], in_=sc[:, o:512], func=Exp, scale=SCALE
                    )
                    if p >= 0:
                        nc.vector.tensor_mul(
                            pt[:, o : o + 128], pt[:, o : o + 128], cmask_sb
                        )
                    vo = 256 * (16 * b + j) + 128 * h
                    nc.tensor.matmul(
                        wv[:, o:512],
                        v_flat[:, vo : vo + 128],
                        pt[:, o:512],
                        start=(j == 0),
                        stop=(j == nj - 1),
                        skip_group_check=True,
                    )
                    cur, lev = pt, 0
                    while zstack and zstack[-1][0] == lev:
                        lev, older = zstack.pop()
                        nc.gpsimd.tensor_add(older, older, cur)
                        cur, lev = older, lev + 1
                    zstack.append((lev, cur))
                while len(zstack) > 1:
                    _, newer = zstack.pop()
                    lev, older = zstack.pop()
                    nc.gpsimd.tensor_add(older, older, newer)
                    zstack.append((lev + 1, older))
                z = zzp.tile([128, 512], f32, tag="zzb", name="z")
                nc.tensor.matmul(
                    z[0:1, :], ones_col, zstack[0][1], start=True, stop=True
                )
                zr = zrp.tile([1, 512], f32r, tag="zr", name="zr")
                with nc.allow_low_precision(
                    reason="f32r is bit-identical to f32"
                ):
                    nc.vector.reciprocal(out=zr, in_=z[0:1, :])
                zb = zzp.tile([128, 512], f32, tag="zzb", name="zb")
                nc.tensor.matmul(zb, ones_row, zr, start=True, stop=True)
                zbs = zbsp.tile([128, 512], f32r, tag="zbs", name="zbs")
                nc.vector.tensor_copy(out=zbs, in_=zb)
                with nc.allow_low_precision(
                    reason="f32r is bit-identical to f32"
                ):
                    nc.vector.tensor_mul(
                        wvn_tiles[(b, h)][:, 512 * ast : 512 * (ast + 1)], wv, zbs
                    )

            def op_(b, ast):
                # out-projection for the 4 token tiles of s-tile ast
                for tk in range(4 * ast, 4 * (ast + 1)):
                    tok0 = S * b + 128 * tk
                    for n in range(4):
                        ops = genp.tile([128, 512], f32, tag="gen", name="opp")
                        for h in range(HPC):
                            nc.tensor.matmul(
                                ops,
                                wvn_tiles[(b, h)][:, 128 * tk : 128 * (tk + 1)],
                                wout_sb[:, h, 512 * n : 512 * (n + 1)],
                                start=(h == 0),
                                stop=(h == HPC - 1),
                            )
                        ost = ostp.tile([128, 512], f32, tag="ost", name="ost")
                        nc.vector.tensor_copy(out=ost, in_=ops)
                        nc.sync.dma_start(
                            out=outp[tok0 : tok0 + 128, 512 * n : 512 * (n + 1)],
                            in_=ost,
                        )

            # pipeline: P1 mega-tile A feeds units (b, ast); out-proj of
            # unit k-1 gives the PE filler work during unit k's exps
            units = [(b, ast) for b in range(B) for ast in range(4)]
            p1(0)
            for i, (b, ast) in enumerate(units):
                if (b, ast) == (0, 1):
                    p1(1)
                elif (b, ast) == (0, 3):
                    p1(2)
                elif (b, ast) == (1, 1):
                    p1(3)
                att(b, 0, ast)
                att(b, 1, ast)
                if i > 0:
                    op_(*units[i - 1])
            op_(*units[-1])

    nc.compile()
    return nc


def _causal_fastpath_ok(mask, cache_pos):
    if cache_pos.shape != (S,) or not np.array_equal(
        np.asarray(cache_pos), np.arange(S, dtype=np.int64).astype(cache_pos.dtype)
    ):
        return False
    m = np.asarray(mask).reshape(S, T)
    rows = np.arange(S)[:, None]
    cols = np.arange(T)[None, :]
    return np.array_equal(m, cols <= rows)


def _numpy_fallback(input_ids, mask, cache_pos, w_qkv, w_out, k_cache, v_cache):
    x = np.asarray(input_ids, dtype=np.float32)
    qkv = np.einsum("bsd,ed->bse", x, np.asarray(w_qkv, np.float32))
    q, k, v = np.split(qkv, 3, axis=-1)

    def heads(t):
        return t.reshape(B, S, H, DH).transpose(0, 2, 1, 3)

    q, k, v = heads(q), heads(k), heads(v)
    kf = np.array(k_cache, np.float32)
    vf = np.array(v_cache, np.float32)
    kf[:, :, np.asarray(cache_pos)] = k
    vf[:, :, np.asarray(cache_pos)] = v
    sc = np.einsum("bhsd,bhtd->bhst", q, kf) * SCALE
    sc = np.where(np.asarray(mask), sc, np.finfo(np.float32).min)
    sc = sc - sc.max(axis=-1, keepdims=True)
    p = np.exp(sc)
    p = p / p.sum(axis=-1, keepdims=True)
    wv = np.einsum("bhst,bhtd->bhsd", p, vf)
    wv = wv.transpose(0, 2, 1, 3).reshape(B, S, NS)
    return np.einsum("bsd,ed->bse", wv, np.asarray(w_out, np.float32))


def _run_on_device(in_maps, trace=False):
    from concourse.bass_utils import run_bass_kernel_spmd

    if "nc" not in _CACHED:
        _CACHED["nc"] = _build_program()
    nc = _CACHED["nc"]
    return run_bass_kernel_spmd(
        nc, in_maps, core_ids=list(range(NCORES)), trace=trace
    )


def _prep_in_maps(input_ids, w_qkv, w_out):
    import ml_dtypes

    bf16 = ml_dtypes.bfloat16
    x2d = np.ascontiguousarray(
        np.asarray(input_ids, np.float32).reshape(TOK, NS).T
    ).astype(bf16)  # [NS, TOK]
    t = np.arange(DH)
    cm = np.concatenate(
        [
            (t[None, :] >= t[:, None]).astype(np.float32),  # attend when s >= t
            np.zeros((DH, 384), np.float32),
            np.ones((DH, 1), np.float32),
        ],
        axis=1,
    ).astype(bf16)
    onesf = np.ones((1, DH), np.float32)
    wq = np.asarray(w_qkv, np.float32)
    wo = np.asarray(w_out, np.float32)
    in_maps = []
    for c in range(NCORES):
        lo, hi = c * DPC, (c + 1) * DPC
        w_slice = np.concatenate(
            [wq[lo:hi], wq[NS + lo : NS + hi], wq[2 * NS + lo : 2 * NS + hi]],
            axis=0,
        )  # [768, NS] (q,k,v rows for this core's heads)
        wT_c = np.ascontiguousarray(w_slice.T).astype(bf16)      # [NS, 768]
        woutT_c = np.ascontiguousarray(wo[:, lo:hi].T)           # [DPC, NS] f32
        in_maps.append(
            {"xT": x2d, "wT": wT_c, "woutT": woutT_c, "cmask": cm, "onesf": onesf}
        )
    return in_maps


def kernel(input_ids, mask, cache_pos, w_qkv, w_out, k_cache, v_cache):
    if not _causal_fastpath_ok(mask, cache_pos):
        return _numpy_fallback(
            input_ids, mask, cache_pos, w_qkv, w_out, k_cache, v_cache
        )
    in_maps = _prep_in_maps(input_ids, w_qkv, w_out)
    res = _run_on_device(in_maps)
    out = np.zeros((TOK, NS), np.float32)
    for r in res.results:
        out += r["outp"]
    return out.reshape(B, S, NS)
